# revision 23
# baseline (speedup 1.0000x reference)
"""SchNet encoder (CASchNetEncoder) distributed Bass kernel for 8 Trainium2 cores.

Strategy:
  - Nodes sharded into 8 contiguous blocks of 5000; each core owns the edges
    whose dst lands in its block (graph partition by destination).
  - Edges with length > cutoff are dropped on the host (exact: their gate C=0).
  - Per core, edges are bucketed by (128-dst-window, src half) and padded to
    a tile schedule that is identical across cores (SPMD: one program).
  - x = h @ lin1.T computed on owned nodes chunk-by-chunk; two AllGathers per
    layer (one per node half, the first at mid-layer) fill two contiguous
    half tables [20480, 128] bf16, double-buffered across layers so the
    next layer's collectives overlap this layer's gathers.  20480 rows per
    half table keeps dma_gather indices inside int16.
  - x[src] fetched with single-row (256B) dma_gather calls of 8 tiles
    (1024 idx), rotating across all 4 SWDGE queues: the Q7 cpu pairs that
    generate descriptors pipeline across queues (measured 2.6 ns/idx vs
    8.2 ns/idx on one queue in isolation).
  - Edge filter W computed in groups of 4 tiles into one PSUM bank (later
    sub-regions ride the first matmul's zero region), then one wide DVE
    multiply W*xg per group.  Scatter-add is two one-hot matmuls per tile
    (W-part and bias-part, lhsT=msg/xg, rhs=fp8 one-hot streamed from DRAM)
    accumulating [FLT, dst] in PSUM -- already the transposed layout the
    node path needs.  The nb2 bias term (with folded log2 shift) is applied
    per window as nb2' * (one-hot @ xg) on the DVE.
  - ShiftedSoftplus log(2) shifts and biases are folded into downstream
    weights/biases on the host.
"""

import numpy as np
import ml_dtypes

from concourse import bass, mybir
import concourse.bacc as bacc
import concourse.tile as tile
from concourse.bass_utils import run_bass_kernel_spmd
from concourse.masks import make_identity

# The activation-table pass picks the first table set containing each
# function: Exp -> exp_and_others, Ln -> natural_log, so a kernel that
# alternates Exp/Ln reloads the LUT on nearly every instruction (~1.3 us
# each).  Steer Exp/Ln/Copy to the one set that holds all three
# (natural_log_exp_and_others) so the table is loaded exactly once.
_COMBINED_SET = "natural_log_exp_and_others"
_STEERED = {
    mybir.ActivationFunctionType.Exp,
    mybir.ActivationFunctionType.Ln,
    mybir.ActivationFunctionType.Copy,
}
if not getattr(bacc, "_act_tables_steered", False):
    _orig_get_tables = bacc.get_activation_tables

    def _steered_get_tables(arch):
        tables = dict(_orig_get_tables(arch))
        return {
            name: (fns if name == _COMBINED_SET else fns - _STEERED)
            for name, fns in tables.items()
        }

    bacc.get_activation_tables = _steered_get_tables
    bacc._act_tables_steered = True

BF16 = mybir.dt.bfloat16
F32 = mybir.dt.float32
I16 = mybir.dt.int16
FP8 = mybir.dt.float8e4

NP_BF16 = ml_dtypes.bfloat16
NP_FP8 = ml_dtypes.float8_e4m3

NCORES = 8
N = 40000
E = 640000
H = 128
FLT = 128
EC = 100
L = 6
IN = 5
CUTOFF = 10.0
LOG2 = float(np.log(2.0))

P = 128
NLOC = N // NCORES            # 5000 nodes per core
NW = (NLOC + P - 1) // P      # 40 destination windows per core
NPAD = NW * P                 # 5120 padded node columns per core
NROWS = NCORES * NPAD         # 40960 rows in the gathered x table
HALF = NPAD // 2              # node rows per half-table block (per core)
HROWS = NCORES * HALF         # 20480 rows per half table (int16-safe)
GMAX = 8                      # max tiles (1024 idx) per dma_gather call
NQ = 4                        # SWDGE queues to rotate gather calls over


def _host_prep(inputs):
    """Partition/sort/pad edges, fold biases, build per-core device arrays."""
    z = np.asarray(inputs["z"], np.float32)
    edge_index = np.asarray(inputs["edge_index"]).astype(np.int64)
    edge_length = np.asarray(inputs["edge_length"], np.float32)
    edge_attr = np.asarray(inputs["edge_attr"], np.float32)

    live = edge_length <= CUTOFF
    src = edge_index[0][live]
    dst = edge_index[1][live]
    attr = edge_attr[live]

    owner = dst // NLOC
    ldst = dst - owner * NLOC
    win = ldst // P
    # gather-table row index: x is AllGathered as two half tables, each
    # [NCORES * HALF, H]; node n of core c sits in half (n >= HALF) at row
    # c * HALF + (n % HALF).  Each half table has 20480 rows (int16-safe).
    nloc_pos = src % NLOC
    seg = (nloc_pos >= HALF).astype(np.int64)
    srow = (src // NLOC) * HALF + (nloc_pos - seg * HALF)
    grp = win * 2 + seg                    # 80 groups per core

    # per (core, group) counts -> shared (max over cores) tile schedule
    NG = NW * 2
    cnt = np.zeros((NCORES, NG), np.int64)
    np.add.at(cnt, (owner, grp), 1)
    tiles_per_grp = -(-cnt.max(axis=0) // P)   # ceil; may be 0
    for w in range(NW):                        # every window needs >=1 tile
        if tiles_per_grp[2 * w] + tiles_per_grp[2 * w + 1] == 0:
            tiles_per_grp[2 * w] = 1
    tstart = np.zeros(NG + 1, np.int64)
    tstart[1:] = np.cumsum(tiles_per_grp)
    ttot = int(tstart[-1])
    ep = ttot * P

    # order by (owner, win, seg, srow): ascending gather rows per group
    order = np.lexsort((srow, seg, win, owner))
    so, sg = owner[order], grp[order]
    sattr = attr[order]
    sldst = ldst[order]
    ssrow = srow[order]
    swin = win[order]
    gkey = so * NG + sg
    gstart_all = np.zeros(NCORES * NG, np.int64)
    np.cumsum(np.bincount(gkey, minlength=NCORES * NG)[:-1], out=gstart_all[1:])
    rank = np.arange(len(so)) - gstart_all[gkey]
    tile_idx = tstart[sg] + rank // P
    part_idx = rank % P
    slot = tile_idx * P + part_idx

    attrT = np.zeros((NCORES, EC, ep), NP_BF16)
    attrT[so, :, slot] = sattr.astype(NP_BF16)

    # gather indices (int16, segment-relative) wrapped for the SWDGE ucode:
    # call-index i of tile t sits at [i%16 (+16k replicas), t*8 + i//16]
    idx16 = np.zeros((NCORES, ep), np.int16)
    idx16[so, slot] = ssrow.astype(np.int16)
    gidx = np.ascontiguousarray(
        np.tile(idx16.reshape(NCORES, ttot * 8, 16).transpose(0, 2, 1),
                (1, 8, 1))
    )  # [NCORES, 128, ttot*8]

    # one-hot scatter tiles: oh[p, t*P + d] = (dst offset of slot (p,t) == d)
    ohall = np.zeros((NCORES, P, ep), NP_FP8)
    ohall[so, part_idx, tile_idx * P + (sldst - swin * P)] = NP_FP8(1)

    # weights with folded shifts
    nW1 = np.asarray(inputs["nn_W1"], np.float32)
    nb1 = np.asarray(inputs["nn_b1"], np.float32)
    nW2 = np.asarray(inputs["nn_W2"], np.float32)
    nb2 = np.asarray(inputs["nn_b2"], np.float32)
    l1W = np.asarray(inputs["lin1_W"], np.float32)
    l2W = np.asarray(inputs["lin2_W"], np.float32)
    l2b = np.asarray(inputs["lin2_b"], np.float32)
    lW = np.asarray(inputs["lin_W"], np.float32)
    lb = np.asarray(inputs["lin_b"], np.float32)
    emblin_W = np.asarray(inputs["emblin_W"], np.float32)
    emblin_b = np.asarray(inputs["emblin_b"], np.float32)

    wx = {
        "nW1T": np.ascontiguousarray(nW1.transpose(0, 2, 1)).astype(NP_BF16),
        "nb1": np.ascontiguousarray(nb1.T),  # [FLT, L] f32
        "nW2T": np.ascontiguousarray(nW2.transpose(0, 2, 1)).astype(NP_BF16),
        "l1WT": np.ascontiguousarray(l1W.transpose(0, 2, 1)).astype(NP_BF16),
        "l2WT": np.ascontiguousarray(l2W.transpose(0, 2, 1)).astype(NP_BF16),
        "l2b": np.ascontiguousarray(l2b.T),  # [H, L] f32
        "lWT": np.ascontiguousarray(lW.transpose(0, 2, 1)).astype(NP_BF16),
        "lbp": np.ascontiguousarray((lb - LOG2 * lW.sum(axis=2)).T),  # [H, L]
        "emblinT": np.ascontiguousarray(emblin_W.T),  # [IN, H] f32
        "nb2colT": np.ascontiguousarray((nb2 - LOG2 * nW2.sum(axis=2)).T),  # [FLT, L]
    }

    featsT = np.zeros((NCORES, IN, NPAD), np.float32)
    ptembT = np.zeros((NCORES, H, NPAD), np.float32)
    for c in range(NCORES):
        blk = z[c * NLOC : (c + 1) * NLOC]
        featsT[c, :, :NLOC] = blk[:, :IN].T
        ptembT[c, :, :NLOC] = blk[:, IN:].T + emblin_b[:, None]

    sched = dict(tstart=tstart, ttot=ttot, ep=ep)
    percore = dict(attrT=attrT, gidx=gidx, ohall=ohall,
                   featsT=featsT, ptembT=ptembT)
    return sched, percore, wx


def _build_program(sched, nchunks=512):
    tstart = sched["tstart"]
    ttot = sched["ttot"]

    EXP = mybir.ActivationFunctionType.Exp
    LN = mybir.ActivationFunctionType.Ln
    CP = mybir.ActivationFunctionType.Copy
    ADD = mybir.AluOpType.add
    MULT = mybir.AluOpType.mult

    nc = bacc.Bacc("TRN2", target_bir_lowering=False, debug=False,
                   enable_asserts=False, num_devices=NCORES,
                   num_swdge_queues=NQ)

    d_attrT = nc.dram_tensor("attrT", [EC, ttot * P], BF16, kind="ExternalInput")
    d_gidx = nc.dram_tensor("gidx", [P, ttot * 8], I16, kind="ExternalInput")
    d_ohall = nc.dram_tensor("ohall", [P, ttot * P], FP8, kind="ExternalInput")
    d_nb2colT = nc.dram_tensor("nb2colT", [FLT, L], F32, kind="ExternalInput")
    d_featsT = nc.dram_tensor("featsT", [IN, NPAD], F32, kind="ExternalInput")
    d_ptembT = nc.dram_tensor("ptembT", [H, NPAD], F32, kind="ExternalInput")
    d_nW1T = nc.dram_tensor("nW1T", [L, EC, FLT], BF16, kind="ExternalInput")
    d_nb1 = nc.dram_tensor("nb1", [FLT, L], F32, kind="ExternalInput")
    d_nW2T = nc.dram_tensor("nW2T", [L, FLT, FLT], BF16, kind="ExternalInput")
    d_l1WT = nc.dram_tensor("l1WT", [L, H, FLT], BF16, kind="ExternalInput")
    d_l2WT = nc.dram_tensor("l2WT", [L, FLT, H], BF16, kind="ExternalInput")
    d_l2b = nc.dram_tensor("l2b", [H, L], F32, kind="ExternalInput")
    d_lWT = nc.dram_tensor("lWT", [L, H, H], BF16, kind="ExternalInput")
    d_lbp = nc.dram_tensor("lbp", [H, L], F32, kind="ExternalInput")
    d_emblinT = nc.dram_tensor("emblinT", [IN, H], F32, kind="ExternalInput")

    d_hout = nc.dram_tensor("hout", [NPAD, H], F32, kind="ExternalOutput")

    d_xlocal = nc.dram_tensor("xlocal", [NPAD, H], BF16, kind="Internal")
    # double-buffered gathered x table, split in two contiguous half tables:
    # layer l reads tab[l%2] while the AllGathers for layer l+1 fill
    # tab[(l+1)%2] concurrently (lo half fires at mid-layer)
    d_xtab = [[nc.dram_tensor(f"xtable{i}{h}", [HROWS, H], BF16,
                              kind="Internal", addr_space="Shared")
               for h in range(2)]
              for i in range(2)]

    # per-window (lo tiles, hi tiles, global tile start)
    wsched = []
    for w in range(NW):
        t0 = int(tstart[2 * w])
        tlo = int(tstart[2 * w + 1] - tstart[2 * w])
        thi = int(tstart[2 * w + 2] - tstart[2 * w + 1])
        wsched.append((t0, tlo, thi))

    qrot = [0]

    with tile.TileContext(nc) as tc:
        with (
            tc.tile_pool(name="const", bufs=1) as cpool,
            tc.tile_pool(name="attr", bufs=4) as p_attr,
            tc.tile_pool(name="xg", bufs=7) as p_xg,
            tc.tile_pool(name="ssp1", bufs=3) as p_ssp,
            tc.tile_pool(name="mx", bufs=6) as p_mx,
            tc.tile_pool(name="oh", bufs=4) as p_oh,
            tc.tile_pool(name="flush", bufs=2) as p_flush,
            tc.tile_pool(name="exp", bufs=2) as p_exp,
            tc.tile_pool(name="pt1", bufs=2, space="PSUM") as p_t1,
            tc.tile_pool(name="pw", bufs=2, space="PSUM") as p_W,
            tc.tile_pool(name="pagg", bufs=2, space="PSUM") as p_agg,
            tc.tile_pool(name="pmisc", bufs=2, space="PSUM") as p_misc,
        ):
            # ---- constants in SBUF ----
            def cload(dram_ap, shape, dt, tag):
                t = cpool.tile(shape, dt, tag=tag)
                nc.sync.dma_start(out=t[:], in_=dram_ap)
                return t

            c_gidx = cload(d_gidx[:], [P, ttot * 8], I16, "gidx")
            c_nb2colT = cload(d_nb2colT[:], [FLT, L], F32, "nb2colT")
            c_nb1 = cload(d_nb1[:], [FLT, L], F32, "nb1")
            c_l2b = cload(d_l2b[:], [H, L], F32, "l2b")
            c_lbp = cload(d_lbp[:], [H, L], F32, "lbp")
            c_emblinT = cload(d_emblinT[:], [IN, H], F32, "emblinT")
            c_featsT = cload(d_featsT[:], [IN, NPAD], F32, "featsT")
            c_ptembT = cload(d_ptembT[:], [H, NPAD], F32, "ptembT")
            c_nW1T = [cload(d_nW1T[l], [EC, FLT], BF16, f"nW1T{l}") for l in range(L)]
            c_nW2T = [cload(d_nW2T[l], [FLT, FLT], BF16, f"nW2T{l}") for l in range(L)]
            c_l1WT = [cload(d_l1WT[l], [H, FLT], BF16, f"l1WT{l}") for l in range(L)]
            c_l2WT = [cload(d_l2WT[l], [FLT, H], BF16, f"l2WT{l}") for l in range(L)]
            c_lWT = [cload(d_lWT[l], [H, H], BF16, f"lWT{l}") for l in range(L)]

            c_ident = cpool.tile([P, P], F32, tag="ident")
            make_identity(nc, c_ident[:])

            hT = cpool.tile([P, NPAD], F32, tag="hT")
            hbf = cpool.tile([P, NPAD], BF16, tag="hbf")
            x_sb = cpool.tile([P, NW, P], BF16, tag="x_sb")
            aggT_sb = cpool.tile([P, NPAD], BF16, tag="aggT_sb")

            nck = NPAD // nchunks   # node-dim chunks
            tpc = nchunks // P      # windows per chunk

            xlocal_v = d_xlocal[:].rearrange("(t p) f -> p t f", p=P)
            hout_v = d_hout[:].rearrange("(t p) f -> p t f", p=P)

            def emit_x_chunk(l, k):
                """x = h @ lin1.T for node chunk k, DMA'd to xlocal."""
                sl = bass.ts(k, nchunks)
                nc.scalar.activation(hbf[:, sl], hT[:, sl], CP)
                for t in range(k * tpc, (k + 1) * tpc):
                    ps = p_misc.tile([P, FLT], F32, tag="misc")
                    nc.tensor.matmul(out=ps[:], lhsT=hbf[:, bass.ts(t, P)],
                                     rhs=c_l1WT[l][:], start=True, stop=True)
                    nc.vector.tensor_copy(out=x_sb[:, t, :], in_=ps[:])
                nc.sync.dma_start(
                    out=xlocal_v[:, k * tpc : (k + 1) * tpc, :],
                    in_=x_sb[:, k * tpc : (k + 1) * tpc, :])

            def emit_collective(l, half):
                sl = slice(0, HALF) if half == 0 else slice(HALF, NPAD)
                nc.gpsimd.collective_compute(
                    "AllGather", mybir.AluOpType.bypass,
                    replica_groups=[list(range(NCORES))],
                    ins=[d_xlocal[sl]], outs=[d_xtab[l % 2][half][:]],
                )

            def emit_gathers(l, w):
                """Single-row gathers for window w, rotating SWDGE queues."""
                xt_lo = d_xtab[l % 2][0][:]
                xt_hi = d_xtab[l % 2][1][:]
                t0, tlo, thi = wsched[w]
                tw = tlo + thi
                xg = p_xg.tile([P, tw, H], BF16, tag="xg")
                for (s0, s1, src_ap) in ((0, tlo, xt_lo), (tlo, tw, xt_hi)):
                    for c0 in range(s0, s1, GMAX):
                        c1 = min(s1, c0 + GMAX)
                        nc.gpsimd.dma_gather(
                            out_ap=xg[:, c0:c1, :],
                            in_ap=src_ap,
                            idxs_ap=c_gidx[:, (t0 + c0) * 8 : (t0 + c1) * 8],
                            num_idxs=(c1 - c0) * P,
                            num_idxs_reg=(c1 - c0) * P,
                            elem_size=H, single_packet=False,
                            queue_num=qrot[0] % NQ)
                        qrot[0] += 1
                return xg

            # ---- h0 = feats @ emblin.T + (ptemb + emblin_b), then x0 ----
            for k in range(nck):
                sl = bass.ts(k, nchunks)
                ps = p_t1.tile([P, nchunks], F32, tag="t1")
                nc.tensor.matmul(out=ps[:], lhsT=c_emblinT[:], rhs=c_featsT[:, sl],
                                 start=True, stop=True)
                nc.vector.tensor_tensor(out=hT[:, sl], in0=ps[:],
                                        in1=c_ptembT[:, sl],
                                        op=mybir.AluOpType.add)
                emit_x_chunk(0, k)
                if k == nck // 2 - 1:
                    emit_collective(0, 0)
            emit_collective(0, 1)

            for l in range(L):
                # ---- edge phase, one destination window at a time ----
                for w in range(NW):
                    t0, tlo, thi = wsched[w]
                    tw = tlo + thi
                    ne = tw * P
                    xg = emit_gathers(l, w)
                    attr_t = p_attr.tile([EC, ne], BF16, tag="attr")
                    nc.sync.dma_start(out=attr_t[:],
                                      in_=d_attrT[:, t0 * P : (t0 + tw) * P])
                    ssp1_t = p_ssp.tile([P, ne], BF16, tag="ssp1")
                    for j in range(0, ne, 512):
                        je = min(ne, j + 512)
                        pt1 = p_t1.tile([P, je - j], F32, tag="t1")
                        nc.tensor.matmul(out=pt1[:], lhsT=c_nW1T[l][:],
                                         rhs=attr_t[:, j:je],
                                         start=True, stop=True)
                        # softplus(x + b) = Ln(Exp(x + b) + 1)
                        ex = p_exp.tile([P, je - j], BF16, tag="exp")
                        nc.scalar.activation(ex[:], pt1[:], EXP,
                                             bias=c_nb1[:, l : l + 1])
                        nc.scalar.activation(ssp1_t[:, j:je], ex[:], LN,
                                             bias=1.0)
                    oh_t = p_oh.tile([P, tw, P], FP8, tag="oh")
                    nc.sync.dma_start(out=oh_t[:],
                                      in_=d_ohall[:, t0 * P : (t0 + tw) * P])
                    pAgg = p_agg.tile([P, 2, P], F32, tag="agg")
                    for g0 in range(0, tw, 4):
                        g1 = min(tw, g0 + 4)
                        gn = g1 - g0
                        # one PSUM bank holds up to 4 tiles' W; the later
                        # sub-regions ride the first matmul's zero region
                        pW = p_W.tile([P, gn, FLT], F32, tag="w")
                        for t in range(g0, g1):
                            nc.tensor.matmul(out=pW[:, t - g0, :],
                                             lhsT=ssp1_t[:, bass.ts(t, P)],
                                             rhs=c_nW2T[l][:],
                                             start=(t == g0), stop=(t == g0),
                                             skip_group_check=(t != g0))
                        mx = p_mx.tile([P, gn, FLT], BF16, tag="mx")
                        nc.vector.tensor_tensor(out=mx[:], in0=pW[:],
                                                in1=xg[:, g0:g1, :], op=MULT)
                        for t in range(g0, g1):
                            # aggT[f, d] += sum_slots mx[slot, f] * oh[slot, d]
                            nc.tensor.matmul(out=pAgg[:, 0, :],
                                             lhsT=mx[:, t - g0, :],
                                             rhs=oh_t[:, t, :],
                                             start=(t == 0), stop=(t == tw - 1))
                            # bias term rides the W-part's zero region
                            # (a second start would wipe its accumulation)
                            nc.tensor.matmul(out=pAgg[:, 1, :], lhsT=xg[:, t, :],
                                             rhs=oh_t[:, t, :],
                                             start=False, stop=False,
                                             skip_group_check=True)
                    # aggT = W-part + nb2' * x-part   (nb2' per partition = FLT)
                    bg = p_mx.tile([P, P], BF16, tag="bg")
                    nc.vector.tensor_scalar(out=bg[:], in0=pAgg[:, 1, :],
                                            scalar1=c_nb2colT[:, l : l + 1],
                                            scalar2=None, op0=MULT)
                    nc.vector.tensor_tensor(out=aggT_sb[:, bass.ts(w, P)],
                                            in0=pAgg[:, 0, :], in1=bg[:],
                                            op=ADD)

                    # ---- node path chunk once its 4 windows are done ----
                    if w % tpc == tpc - 1:
                        k = w // tpc
                        sl = bass.ts(k, nchunks)
                        p2 = p_t1.tile([P, nchunks], F32, tag="t1")
                        nc.tensor.matmul(out=p2[:], lhsT=c_l2WT[l][:],
                                         rhs=aggT_sb[:, sl],
                                         start=True, stop=True)
                        ex2 = p_exp.tile([P, nchunks], F32, tag="exp")
                        nc.scalar.activation(ex2[:], p2[:], EXP,
                                             bias=c_l2b[:, l : l + 1])
                        s_t = p_mx.tile([P, nchunks], BF16, tag="s")
                        nc.scalar.activation(s_t[:], ex2[:], LN,
                                             bias=1.0)
                        p3 = p_W.tile([P, nchunks], F32, tag="w")
                        nc.tensor.matmul(out=p3[:], lhsT=c_lWT[l][:],
                                         rhs=s_t[:], start=True, stop=True)
                        # h += p3 + lbp  (fused: (p3 + lbp) + hT)
                        nc.vector.scalar_tensor_tensor(
                            out=hT[:, sl], in0=p3[:],
                            scalar=c_lbp[:, l : l + 1], in1=hT[:, sl],
                            op0=ADD, op1=ADD)
                        if l + 1 < L:
                            emit_x_chunk(l + 1, k)
                            if k == nck // 2 - 1:
                                emit_collective(l + 1, 0)
                if l + 1 < L:
                    emit_collective(l + 1, 1)

            # ---- output: transpose hT back to [node, feat] ----
            for t in range(NW):
                pt = p_misc.tile([P, P], F32, tag="misc")
                nc.tensor.transpose(out=pt[:], in_=hT[:, bass.ts(t, P)],
                                    identity=c_ident[:])
                ob = p_attr.tile([P, P], F32, tag="ob")
                nc.vector.tensor_copy(out=ob[:], in_=pt[:])
                nc.sync.dma_start(out=hout_v[:, t, :], in_=ob[:])

    nc.compile()
    return nc


def kernel(**inputs):
    sched, percore, wx = _host_prep(inputs)
    nc = _build_program(sched)

    in_maps = []
    for c in range(NCORES):
        m = {k: np.ascontiguousarray(percore[k][c]) for k in percore}
        for k, v in wx.items():
            m[k] = v
        in_maps.append(m)

    res = run_bass_kernel_spmd(nc, in_maps, core_ids=list(range(NCORES)))
    out = np.empty((N, H), np.float32)
    for c in range(NCORES):
        out[c * NLOC : (c + 1) * NLOC] = res.results[c]["hout"][:NLOC]
    return out
